# revision 72
# baseline (speedup 1.0000x reference)
"""Additive attention (B=4, Q=256, KV=1024, H=128, VS=256) on 8 Trainium2 cores.

Sharding: each core processes 32 query rows of every batch (4 groups of 32
row-slots).  Per batch, only a KV prefix of width ceil32(valid_len) is
computed; masked columns contribute exactly 0 to the softmax, so skipping
them is exact.  No collectives are needed.  The program is specialized per
valid_lens configuration at call time and cached.

v2 dataflow per core (small single-chunk group first, smallest last):
  PE    : k/q projections in full fp32 (accuracy-critical, pre-tanh)
  DVE   : sums[h, kv] = kp[h, kv] + qp[h, s]   (fp32 tensor_scalar add, 2x)
  ACT   : tanh over 8-row batches, fp32 in -> fp16 feats out; ACT is the
          throughput floor (1 elem/lane/cycle @1.2GHz) and is kept saturated
          by software-pipelined emission (adds 2 sub-batches ahead,
          projections a full group ahead, copies split off the add path)
  PE    : per-group PSUM score tiles (bank-padded); mask written first via
          one K=4 matmul (ind outer mask); score rows accumulate via fp16
          one-hot matmuls (single pass, wv fp16, 4B-aligned even/odd window
          tables); zero-accumulate dummy matmuls keep the PE HAM clock warm;
          fp16 probs transpose; attn @ V in fp16 (V converted on host)
  ACT/DVE: per-group row-max + exp(accum_out row sums) + reciprocal + scale;
          per-group tails interleave into the next group's compute.
"""
import math
import os
import sys

import numpy as np

for _p in ("/opt/trn_rl_repo", "/root/.axon_site/_ro/trn_rl_repo"):
    if os.path.isdir(_p):
        if _p not in sys.path:
            sys.path.insert(0, _p)
        break

B, Q, KV, QS, KS, H, VS = 4, 256, 1024, 128, 128, 128, 256
P = 128
N_CORES = 8
GROUP_ROWS = 32          # rows per (core, batch)
SUB = 8                  # rows per tanh batch

PROFILE = False          # set by test.py; enables NTFF tracing
LAST_RESULTS = None
SIMULATE = False         # set by test.py; run CoreSim instead of hardware
LAST_EXEC_NS = None

_prog_cache = {}


def _v2_fits(Ws, l0flags):
    if any(l0flags):
        return False
    banks = sum((w + 511) // 512 for w in Ws)
    return banks <= 7


def _build_v2(cfg):
    """Per-group PSUM score tiles, fp16 single-pass one-hot score reduction,
    interleaved per-group softmax/attnV tails."""
    Ws, _l0 = cfg
    import contextlib

    import concourse.bacc as bacc
    import concourse.mybir as mybir
    import concourse.tile as tile

    f32 = mybir.dt.float32
    f16 = mybir.dt.float16
    W = list(Ws)                         # per-group computed KV width
    Wmax = max(W)
    sumW = sum(W)
    # group 0 packs last so its kp is adjacent to the qp scratch columns
    offs = [sumW - W[0]] + [sum(W[1:g]) for g in range(1, B)]
    nc = bacc.Bacc("TRN2", target_bir_lowering=False, debug=False,
                   enable_asserts=True, num_devices=N_CORES)

    # f16 blob layout (all segment starts 4B-aligned):
    #   tblA [0:256)   wv at col 128 -> window [128-s, 256-s) for even s
    #   tblB [256:512) wv at col 127+256 -> window [127-s, 255-s) for odd s
    #   ind  [512:640)
    #   mask [640:640+Wmax)
    #   ident16 [640+Wmax : 768+Wmax)
    NB16 = 768 + Wmax
    blob32_d = nc.dram_tensor("blob32", [P, 3 * P], f32,
                              kind="ExternalInput").ap()
    blob16_d = nc.dram_tensor("blob16", [P, NB16], f16,
                              kind="ExternalInput").ap()
    kT_d = nc.dram_tensor("kT", [P, sumW], f32,
                          kind="ExternalInput").ap()
    V_d = nc.dram_tensor("V", [B, KV, VS], f16, kind="ExternalInput").ap()
    out_d = nc.dram_tensor("out", [P, VS], f32, kind="ExternalOutput").ap()

    with tile.TileContext(nc) as tc, contextlib.ExitStack() as ctx:
        const = ctx.enter_context(tc.tile_pool(name="const", bufs=1))
        ktp = ctx.enter_context(tc.tile_pool(name="ktp", bufs=3))
        sums_kb = SUB * Wmax * 4 / 1024
        n_sums = max(2, min(4, int(72 // sums_kb) or 2))
        sums_pool = ctx.enter_context(
            tc.tile_pool(name="sumsp", bufs=n_sums))
        feats_pool = ctx.enter_context(
            tc.tile_pool(name="featsp", bufs=3))
        ptp = ctx.enter_context(tc.tile_pool(name="ptp", bufs=6))
        psum = ctx.enter_context(tc.tile_pool(name="psum", bufs=1,
                                              space="PSUM"))

        # ---- constants: one fp32 blob (sync queue) + one fp16 blob
        # (gpsimd queue) ----
        blob32 = const.tile([P, 3 * P], f32)
        nc.sync.dma_start(blob32[:], blob32_d[:])
        qt_sb = blob32[:, 0:P]
        wq_sb = blob32[:, P:2 * P]
        wk_sb = blob32[:, 2 * P:3 * P]
        blob16 = const.tile([P, NB16], f16)
        nc.gpsimd.dma_start(blob16[:], blob16_d[:])
        tblA = blob16[:, 0:256]
        tblB = blob16[:, 256:512]
        ind_sb = blob16[0:B, 512:512 + P]
        mask_sb = blob16[0:B, 640:640 + Wmax]
        ident_sb = blob16[:, 640 + Wmax:768 + Wmax]

        def wv_window(s):
            # [128, 128] one-hot slice with wv at column s, 4B-aligned start
            if s % 2 == 0:
                return tblA[:, 128 - s: 256 - s]
            return tblB[:, 127 - s: 255 - s]

        kp_sb = const.tile([P, sumW + P], f32)
        qp_sb = const.tile([P, P], f32)
        probs = const.tile([P, Wmax], f16)
        rowsum = const.tile([P, 1], f32)
        rinv = const.tile([P, 1], f32)
        out_sb = const.tile([P, VS], f32)
        nrowmax = const.tile([P, 1], f32)
        nc.gpsimd.memset(probs[:], 0.0)

        # ---- per-group PSUM score tiles (bank-padded) + output tile ----
        scores = [psum.tile([P, 512 * ((W[g] + 511) // 512)], f32,
                            name=f"scores_{g}") for g in range(B)]
        out_ps = psum.tile([P, 512], f32, name="out_ps")
        ptps = ctx.enter_context(tc.tile_pool(name="ptps", bufs=1,
                                              space="PSUM"))
        pt_ps4 = ptps.tile([P, 512], f16, name="pt_ps4")
        vts = {}

        def chunks(w):
            return [(c0, min(c0 + 512, w)) for c0 in range(0, w, 512)]

        def proj_group_mm(g):
            for c0, c1 in chunks(W[g]):
                n = c1 - c0
                kt_t = ktp.tile([P, 512], f32, tag="kt",
                                name=f"kt_{g}_{c0}")
                nc.sync.dma_start(kt_t[:, :n], kT_d[:, offs[g] + c0:
                                                    offs[g] + c1])
                nc.tensor.matmul(scores[g][:, c0:c1], wk_sb[:], kt_t[:, :n],
                                 start=True, stop=True,
                                 skip_group_check=True)
            for c in range((W[g] + P - 1) // P):
                vts[(g, c)] = const.tile([P, VS], f16, name=f"v_{g}_{c}")
                nc.gpsimd.dma_start(vts[(g, c)][:],
                                    V_d[g, c * P:(c + 1) * P, :])

        def proj_group_copy(g, extra=0):
            for ci, (c0, c1) in enumerate(chunks(W[g])):
                e = extra if ci == 0 else 0
                nc.vector.tensor_copy(
                    kp_sb[:, offs[g] + c0: offs[g] + c1 + e],
                    scores[g][:, c0:c1 + e])
            emit_masks(g)

        def tail_pieces(g):
            wg = W[g]
            band = slice(GROUP_ROWS * g, GROUP_ROWS * (g + 1))

            def piece0():
                nc.vector.reduce_max(nrowmax[band], scores[g][band, :wg],
                                     axis=mybir.AxisListType.X, negate=True)
                nc.scalar.activation(probs[band, :wg], scores[g][band, :wg],
                                     mybir.ActivationFunctionType.Exp,
                                     bias=nrowmax[band, 0:1], scale=1.0,
                                     accum_out=rowsum[band, 0:1])
                nc.vector.reciprocal(rinv[band], rowsum[band])

            ncg = (wg + P - 1) // P

            def tc_chunks(cs):
                for c in cs:
                    cw = min(P, wg - c * P)
                    pslot = (c % 4) * P
                    nc.tensor.transpose(pt_ps4[:cw, pslot:pslot + P],
                                        probs[:, c * P:c * P + cw],
                                        ident_sb[:])
                    pt_sb = ptp.tile([P, P], f16, tag="pt",
                                     name=f"pt_{g}_{c}")
                    nc.vector.tensor_copy(pt_sb[:cw, :],
                                          pt_ps4[:cw, pslot:pslot + P])
                    vts[(g, c, "pt")] = pt_sb

            def av_chunks(cs):
                for c in cs:
                    gw = min(P, wg - c * P)
                    nc.tensor.matmul(
                        out_ps[band, 0:VS],
                        vts[(g, c, "pt")][:gw, band],
                        vts[(g, c)][:gw, :],
                        start=(c == 0), stop=(c == ncg - 1),
                        tile_position=(0, GROUP_ROWS * g),
                        skip_group_check=True)

            def piece1():
                tc_chunks(range(0, (ncg + 1) // 2))

            def piece2():
                tc_chunks(range((ncg + 1) // 2, ncg))
                av_chunks(range(0, (ncg + 1) // 2))

            def piece3():
                av_chunks(range((ncg + 1) // 2, ncg))
                nc.vector.tensor_scalar_mul(out_sb[band, :],
                                            out_ps[band, 0:VS],
                                            rinv[band, 0:1])
                nc.sync.dma_start(out_d[band, :], out_sb[band, :])

            return [piece0, piece1, piece2, piece3]

        def emit_masks(g):
            for c0, c1 in chunks(W[g]):
                nc.tensor.matmul(scores[g][:, c0:c1], ind_sb[:],
                                 mask_sb[:, c0:c1],
                                 start=True, stop=False,
                                 skip_group_check=True)

        def emit_adds(g, sb):
            wg = W[g]
            sums = sums_pool.tile([P, SUB * wg], f32, tag="sums",
                                  name=f"sums_{g}_{sb}")
            for j in range(SUB):
                s = GROUP_ROWS * g + SUB * sb + j
                nc.vector.tensor_scalar_add(
                    sums[:, j * wg:(j + 1) * wg],
                    kp_sb[:, offs[g]: offs[g] + wg],
                    qp_src[:, s: s + 1])
            return sums

        def emit_compute(g, sb, sums, split):
            wg = W[g]
            feats = feats_pool.tile([P, SUB * wg], f16, tag="feats",
                                    name=f"feats_{g}_{sb}")
            last_row = (sb == GROUP_ROWS // SUB - 1)
            halves = ([(0, SUB // 2), (SUB // 2, SUB)] if split
                      else [(0, SUB)])
            for j0, j1 in halves:
                nc.scalar.activation(feats[:, j0 * wg:j1 * wg],
                                     sums[:, j0 * wg:j1 * wg],
                                     mybir.ActivationFunctionType.Tanh)
                for j in range(j0, j1):
                    s = GROUP_ROWS * g + SUB * sb + j
                    for c0, c1 in chunks(wg):
                        nc.tensor.matmul(
                            scores[g][:, c0:c1],
                            wv_window(s),
                            feats[:, j * wg + c0: j * wg + c1],
                            start=False,
                            stop=(last_row and j == SUB - 1),
                            skip_group_check=True)
            if not last_row:
                # zero-accumulate dummies: keep the PE HAM busy-window warm
                # (tblA[:, 0:128] is all zeros; adds exactly 0 to scores)
                dw = min(512, wg)
                for _ in range(4 if wg > 512 else 2):
                    nc.tensor.matmul(scores[g][:, 0:dw], tblA[:, 0:P],
                                     feats[:, 0:dw], start=False,
                                     stop=False, skip_group_check=True)

        # software-pipelined emission: each slot's adds enter the DVE queue
        # LA sub-batches ahead of its tanh/matmuls
        LA = min(2, n_sums - 1)
        NSB = GROUP_ROWS // SUB
        slots = [(g, sb) for g in range(B) for sb in range(NSB)]
        pending = []
        sums_store = {}

        add_ptr = [0]

        def pump_adds(limit):
            while add_ptr[0] <= min(limit, len(slots) - 1):
                sl = slots[add_ptr[0]]
                sums_store[sl] = emit_adds(*sl)
                add_ptr[0] += 1

        proj_group_mm(0)
        # q projection emitted after proj(0)'s matmuls so the first kp chunk
        # isn't stuck behind the cold qp matmul in the PE queue.  When the
        # first group is narrow, qp parks right after its kp in the same
        # PSUM bank and both copy out in a single DVE op (into kp_sb's
        # scratch columns beyond sumW); else it borrows the last group's
        # score bank.
        if W[0] <= 384:
            nc.tensor.matmul(scores[0][:, W[0]:W[0] + P], wq_sb[:],
                             qt_sb[:], start=True, stop=True,
                             skip_group_check=True)
            proj_group_copy(0, extra=P)
            qp_src = kp_sb[:, sumW:sumW + P]
        else:
            nc.tensor.matmul(scores[B - 1][:, 0:P], wq_sb[:], qt_sb[:],
                             start=True, stop=True, skip_group_check=True)
            nc.vector.tensor_copy(qp_sb[:], scores[B - 1][:, 0:P])
            proj_group_copy(0)
            qp_src = qp_sb[:, 0:P]
        pump_adds(LA - 1)
        for i, (g, sb) in enumerate(slots):
            if sb == 0 and g + 1 < B:
                # projection DMA+matmuls a full group ahead (no buffer
                # constraint); the DVE copies follow one slot later so they
                # don't head-block the adds in the DVE queue
                proj_group_mm(g + 1)
            if sb == 2 and g + 1 < B:
                proj_group_copy(g + 1)
            for _ in range(2 if g == B - 1 else 1):
                if pending:
                    pending.pop(0)()
            pump_adds(i + LA)
            emit_compute(g, sb, sums_store.pop((g, sb)), split=(i == 0))
            if sb == NSB - 1:
                pending.extend(tail_pieces(g))
        for piece in pending:
            piece()

    nc.compile()
    return nc


def _build_v1(cfg):
    """Generic fallback (original baseline): fp32r 2-pass one-hot scores,
    shared score tile, trailing softmax."""
    Ws, l0flags = cfg
    ncfg = [(w + P - 1) // P for w in Ws]
    import contextlib

    import concourse.bacc as bacc
    import concourse.mybir as mybir
    import concourse.tile as tile
    from concourse.tile_rust import add_dep_helper

    f32 = mybir.dt.float32
    W = list(Ws)                        # per-group computed KV width
    Wmax = W[0]
    nc = bacc.Bacc("TRN2", target_bir_lowering=False, debug=False,
                   enable_asserts=True, num_devices=N_CORES)

    blob_d = nc.dram_tensor("blob", [P, 3 * P], f32,
                            kind="ExternalInput").ap()
    ident_d = nc.dram_tensor("ident", [P, P], f32,
                             kind="ExternalInput").ap()
    wvdb_d = nc.dram_tensor("wvdb", [P, 2 * (2 * P - 1)], mybir.dt.float32r,
                            kind="ExternalInput").ap()
    kT_d = nc.dram_tensor("kT", [P, B * KV], f32, kind="ExternalInput").ap()
    V_d = nc.dram_tensor("V", [B, KV, VS], f32, kind="ExternalInput").ap()
    ind_d = nc.dram_tensor("ind", [B, P], mybir.dt.float32r,
                           kind="ExternalInput").ap()
    wvd0_d = nc.dram_tensor("wvd0", [P, 2 * P - 1], mybir.dt.float32r,
                            kind="ExternalInput").ap()
    mask_d = nc.dram_tensor("mask", [B, Wmax], mybir.dt.float32r,
                            kind="ExternalInput").ap()
    out_d = nc.dram_tensor("out", [P, VS], f32, kind="ExternalOutput").ap()

    with tile.TileContext(nc) as tc, contextlib.ExitStack() as ctx:
        const = ctx.enter_context(tc.tile_pool(name="const", bufs=1))
        ktp = ctx.enter_context(tc.tile_pool(name="ktp", bufs=2))
        vbytes = sum((w + P - 1) // P for w in W)          # V tiles, KB/part
        feats_kb = SUB * Wmax * 4 / 1024
        feats_bufs = max(2, min(5, int((192 - 50 - vbytes - 16) // feats_kb)))
        feats_pool = ctx.enter_context(
            tc.tile_pool(name="featsp", bufs=feats_bufs))
        small = ctx.enter_context(tc.tile_pool(name="small", bufs=1))
        psum = ctx.enter_context(tc.tile_pool(name="psum", bufs=1, space="PSUM"))
        psum2 = ctx.enter_context(tc.tile_pool(name="psum2", bufs=1, space="PSUM"))

        f32r = mybir.dt.float32r
        blob = const.tile([P, 3 * P], f32)
        nc.sync.dma_start(blob[:], blob_d[:])
        qt_sb = blob[:, 0:P]
        wq_sb = blob[:, P:2 * P]
        wk_sb = blob[:, 2 * P:3 * P]
        ident_t = const.tile([P, P], f32)
        nc.gpsimd.dma_start(ident_t[:], ident_d[:])
        ident_sb = ident_t[:]
        wvdb = const.tile([P, 2 * (2 * P - 1)], f32r)
        nc.gpsimd.dma_start(wvdb[:], wvdb_d[:])
        wvd_hi = wvdb[:, 0:2 * P - 1]
        wvd_lo = wvdb[:, 2 * P - 1:]
        if any(l0flags):
            wvd0_t = const.tile([P, 2 * P - 1], f32r)
            nc.gpsimd.dma_start(wvd0_t[:], wvd0_d[:])
            wvd0 = wvd0_t[:]
        ind_sb = const.tile([B, P], f32r)
        nc.gpsimd.dma_start(ind_sb[:], ind_d[:])
        mask_sb = const.tile([B, Wmax], f32r)
        nc.gpsimd.dma_start(mask_sb[:], mask_d[:])

        scores_ps = psum.tile([P, Wmax], f32, name="scores_ps")
        vts = {}

        qp_ps = psum2.tile([P, P], f32, tag="ptqp", bufs=3, name="qp_ps")
        nc.tensor.matmul(qp_ps[:], wq_sb[:], qt_sb[:], start=True, stop=True)
        qp_sb = const.tile([P, P], f32)
        nc.vector.tensor_copy(qp_sb[:], qp_ps[:])

        kp_sb = const.tile([P, B * KV], f32)
        g_order = sorted(range(B), key=lambda g: W[g])

        def proj_group(g):
            cp = None
            for j in range(0, W[g], 512):
                n = min(512, W[g] - j)
                kt_t = ktp.tile([P, 512], f32, tag="kt", name=f"kt_{g}_{j}",
                                bufs=4)
                nc.sync.dma_start(kt_t[:, :n], kT_d[:, g * KV + j: g * KV + j + n])
                kp_ps = psum2.tile([P, 512], f32, tag="proj", bufs=2,
                                   name=f"kp_ps_{g}_{j}")
                nc.tensor.matmul(kp_ps[:, :n], wk_sb[:], kt_t[:, :n],
                                 start=True, stop=True)
                cp = nc.vector.tensor_copy(
                    kp_sb[:, g * KV + j: g * KV + j + n], kp_ps[:, :n])
            return cp

        for c0 in range(0, Wmax, 512):
            c1 = min(c0 + 512, Wmax)
            nc.tensor.matmul(scores_ps[:, c0:c1], ind_sb[:], mask_sb[:, c0:c1],
                             start=True, stop=False, skip_group_check=True)

        for gi, g in enumerate(g_order):
            wg = W[g]
            last_kp_copy = proj_group(g)
            wsrcs = [wvd0] if l0flags[g] else [wvd_hi, wvd_lo]
            for sb in range(GROUP_ROWS // SUB):
                feats = feats_pool.tile([P, SUB * wg], f32r, tag="feats",
                                        name=f"feats_{g}_{sb}")
                for j in range(SUB):
                    s = GROUP_ROWS * g + SUB * sb + j
                    nc.vector.tensor_scalar_add(
                        feats[:, j * wg:(j + 1) * wg],
                        kp_sb[:, g * KV: g * KV + wg],
                        qp_sb[:, s: s + 1])
                nc.scalar.activation(feats[:], feats[:],
                                     mybir.ActivationFunctionType.Tanh)
                for j in range(SUB):
                    s = GROUP_ROWS * g + SUB * sb + j
                    last = (gi == B - 1 and sb == GROUP_ROWS // SUB - 1
                            and j == SUB - 1)
                    for wsrc in wsrcs:
                        for c0 in range(0, wg, 512):
                            c1 = min(c0 + 512, wg)
                            nc.tensor.matmul(
                                scores_ps[:, c0:c1],
                                wsrc[:, P - 1 - s: 2 * P - 1 - s],
                                feats[:, j * wg + c0: j * wg + c1],
                                start=False,
                                stop=(last and wsrc is wsrcs[-1]
                                      and c0 + 512 >= wg),
                                skip_group_check=True)

        for g in range(B):
            for c in range((W[g] + P - 1) // P):
                vts[(g, c)] = const.tile([P, VS], f32, name=f"v_{g}_{c}")
                vdma = nc.sync.dma_start(vts[(g, c)][:],
                                         V_d[g, c * P:(c + 1) * P, :])
                add_dep_helper(vdma.ins, last_kp_copy.ins,
                               reason="V after kp: kT wins head HBM bw")

        nrowmax = small.tile([P, 1], f32)
        nc.vector.reduce_max(nrowmax[:], scores_ps[:, :Wmax],
                             axis=mybir.AxisListType.X, negate=True)
        probs = small.tile([P, Wmax], f32)
        n_ec = (Wmax + 255) // 256
        psums = small.tile([P, n_ec], f32)
        for e in range(n_ec):
            e0, e1 = e * 256, min((e + 1) * 256, Wmax)
            nc.scalar.activation(probs[:, e0:e1], scores_ps[:, e0:e1],
                                 mybir.ActivationFunctionType.Exp,
                                 bias=nrowmax[:, 0:1], scale=1.0,
                                 accum_out=psums[:, e:e + 1])
        rowsum = small.tile([P, 1], f32)
        nc.vector.reduce_sum(rowsum[:], psums[:], axis=mybir.AxisListType.X)
        rinv = small.tile([P, 1], f32)
        nc.vector.reciprocal(rinv[:], rowsum[:])

        out_ps = psum.tile([P, VS], f32, name="out_ps")
        for c in range(ncfg[0]):
            cw = min(P, Wmax - c * P)
            pt_ps = psum2.tile([P, P], f32, tag="ptqp", bufs=3,
                               name=f"pt_ps{c}")
            nc.tensor.transpose(pt_ps[:cw, :], probs[:, c * P:c * P + cw],
                                ident_sb[:])
            pt_sb = small.tile([P, P], f32, name=f"pt_sb{c}")
            nc.vector.tensor_copy(pt_sb[:cw, :], pt_ps[:cw, :])
            for g in range(B):
                if c * P < W[g]:
                    gw = min(P, W[g] - c * P)
                    nc.tensor.matmul(
                        out_ps[GROUP_ROWS * g: GROUP_ROWS * (g + 1), :],
                        pt_sb[:gw, GROUP_ROWS * g: GROUP_ROWS * (g + 1)],
                        vts[(g, c)][:gw, :],
                        start=(c == 0), stop=(c == (W[g] + P - 1) // P - 1),
                        tile_position=(0, GROUP_ROWS * g),
                        skip_group_check=True)

        out_sb = small.tile([P, VS], f32)
        nc.vector.tensor_scalar_mul(out_sb[:], out_ps[:], rinv[:, 0:1])
        nc.sync.dma_start(out_d[:], out_sb[:])

    nc.compile()
    return nc


def _get_program(key, builder):
    if key not in _prog_cache:
        _prog_cache[key] = builder(key[1])
    return _prog_cache[key]


def _run(nc, in_maps):
    global LAST_EXEC_NS, LAST_RESULTS
    if SIMULATE:
        from concourse.bass_interp import CoreSim
        outs = []
        for c in range(N_CORES):
            sim = CoreSim(nc, trace=False)
            for name, v in in_maps[c].items():
                sim.tensor(name)[:] = v
            sim.simulate(check_with_hw=False)
            outs.append(sim.tensor("out").copy())
        return outs
    from concourse import bass_utils
    kw = {"trace": True} if PROFILE else {}
    res = bass_utils.run_bass_kernel_spmd(nc, in_maps, list(range(N_CORES)),
                                          **kw)
    if PROFILE:
        LAST_EXEC_NS = res.exec_time_ns
        LAST_RESULTS = res
    return [res.results[c]["out"] for c in range(N_CORES)]


def kernel(queries, keys, values, valid_lens, Wq, Wk, wv):
    queries = np.ascontiguousarray(np.asarray(queries), dtype=np.float32)
    keys = np.ascontiguousarray(np.asarray(keys), dtype=np.float32)
    values = np.ascontiguousarray(np.asarray(values), dtype=np.float32)
    Wq = np.ascontiguousarray(np.asarray(Wq), dtype=np.float32)
    Wk = np.ascontiguousarray(np.asarray(Wk), dtype=np.float32)
    wv = np.ascontiguousarray(np.asarray(wv), dtype=np.float32)
    vl = [int(x) for x in np.asarray(valid_lens)]

    def width_v2(L):
        if L <= 0:
            return KV
        return min(KV, max(32, 8 * math.ceil(L / 8)))

    W_b = [width_v2(L) for L in vl]
    order = sorted(range(B), key=lambda b: (-W_b[b], b))
    if B >= 3:
        # small single-chunk group first (fast pipeline start), smallest last
        order = [order[-2]] + order[:-2] + [order[-1]]
    Ws = tuple(W_b[b] for b in order)
    l0flags = tuple(vl[order[g]] == 0 for g in range(B))

    if _v2_fits(Ws, l0flags):
        return _kernel_v2(queries, keys, values, vl, Wq, Wk, wv, order, Ws)
    return _kernel_v1(queries, keys, values, vl, Wq, Wk, wv)


def _kernel_v2(queries, keys, values, vl, Wq, Wk, wv, order, Ws):
    Wmax = max(Ws)
    sumW = sum(Ws)
    offs = [sum(Ws[:g]) for g in range(B)]
    nc = _get_program(("v2", (Ws, ())), _build_v2)

    kT = np.concatenate(
        [keys[order[g]].T[:, :Ws[g]] for g in list(range(1, B)) + [0]],
        axis=1)
    kT = np.ascontiguousarray(kT)                        # [128, sumW]
    f16 = np.float16
    Vm = np.ascontiguousarray(
        np.stack([values[order[g]] for g in range(B)]).astype(f16))

    blob16 = np.zeros((P, 768 + Wmax), f16)
    blob16[:, 128] = wv.astype(f16)          # tblA: window [128-s,256-s), s even
    blob16[:, 256 + 127] = wv.astype(f16)    # tblB: window [127-s,255-s), s odd
    for g in range(B):
        blob16[g, 512 + GROUP_ROWS * g: 512 + GROUP_ROWS * (g + 1)] = 1.0
        L = vl[order[g]]
        blob16[g, 640 + min(L, Wmax): 640 + Wmax] = -30000.0
    blob16[:, 640 + Wmax: 768 + Wmax] = np.eye(P, dtype=f16)

    blob32 = np.zeros((P, 3 * P), np.float32)
    blob32[:, P:2 * P] = Wq
    blob32[:, 2 * P:3 * P] = Wk
    shared = {"kT": kT, "V": Vm, "blob16": blob16}
    in_maps = []
    for c in range(N_CORES):
        qT = np.concatenate(
            [queries[order[g], c * GROUP_ROWS:(c + 1) * GROUP_ROWS, :].T
             for g in range(B)], axis=1)
        bl = blob32.copy()
        bl[:, 0:P] = qT
        m = dict(shared)
        m["blob32"] = bl
        in_maps.append(m)

    outs = _run(nc, in_maps)
    out = np.zeros((B, Q, VS), np.float32)
    for c in range(N_CORES):
        for g in range(B):
            out[order[g], c * GROUP_ROWS:(c + 1) * GROUP_ROWS, :] = \
                outs[c][GROUP_ROWS * g: GROUP_ROWS * (g + 1), :]
    return out


def _kernel_v1(queries, keys, values, vl, Wq, Wk, wv):
    def width(L):
        # fp32r matmul chunks must be >= 256 cols; widths are 32-multiples
        if L <= 0:
            return KV
        L = min(L, KV)
        if L <= 512:
            return max(256, 32 * math.ceil(L / 32))
        return 512 + max(256, 32 * math.ceil((L - 512) / 32))

    W_b = [width(L) for L in vl]
    order = sorted(range(B), key=lambda b: (-W_b[b], b))
    Ws = tuple(W_b[b] for b in order)
    l0flags = tuple(vl[order[g]] == 0 for g in range(B))
    Wmax = Ws[0]

    nc = _get_program(("v1", (Ws, l0flags)), _build_v1)

    kT = np.concatenate([keys[order[g]].T for g in range(B)], axis=1)
    kT = np.ascontiguousarray(kT)                        # [128, 4096]
    Vm = np.ascontiguousarray(np.stack([values[order[g]] for g in range(B)]))
    ind = np.zeros((B, P), np.float32)
    for g in range(B):
        ind[g, GROUP_ROWS * g: GROUP_ROWS * (g + 1)] = 1.0
    mask = np.full((B, Wmax), -1e6, np.float32)
    for g in range(B):
        L = vl[order[g]]
        if L > 0:
            mask[g, :min(L, Wmax)] = 0.0
        else:
            mask[g, :] = 0.0
    ident = np.eye(P, dtype=np.float32)

    import ml_dtypes
    bf16 = ml_dtypes.bfloat16
    wv_hi = wv.astype(bf16).astype(np.float32)
    DW = 2 * P - 1
    blob = np.zeros((P, 3 * P), np.float32)
    blob[:, P:2 * P] = Wq
    blob[:, 2 * P:3 * P] = Wk
    wvdb = np.zeros((P, 2 * DW), np.float32)
    wvdb[:, P - 1] = wv_hi
    wvdb[:, DW + P - 1] = wv - wv_hi
    wvd0 = np.zeros((P, DW), np.float32)
    shared = {"kT": kT, "V": Vm, "ind": ind, "mask": mask, "wvd0": wvd0,
              "wvdb": wvdb, "ident": ident}
    in_maps = []
    for c in range(N_CORES):
        qT = np.concatenate(
            [queries[order[g], c * GROUP_ROWS:(c + 1) * GROUP_ROWS, :].T
             for g in range(B)], axis=1)
        bl = blob.copy()
        bl[:, 0:P] = qT
        m = dict(shared)
        m["blob"] = bl
        in_maps.append(m)

    outs = _run(nc, in_maps)
    out = np.zeros((B, Q, VS), np.float32)
    for c in range(N_CORES):
        for g in range(B):
            out[order[g], c * GROUP_ROWS:(c + 1) * GROUP_ROWS, :] = \
                outs[c][GROUP_ROWS * g: GROUP_ROWS * (g + 1), :]
    return out
